# revision 1
# baseline (speedup 1.0000x reference)
"""Trainium2 Bass kernel for CombinedSurvLoss (NLL survival + pairwise rank loss).

Strategy (8-core SPMD, row-sharded rank matrix):
  - risk[j] = -sum(cumprod(1-sigmoid(outputs[j]))); e[j] = exp(risk[j]).
    Since risk in (-4, 0), exp never overflows, so the masked logsumexp
    lse[i] = logsumexp_{j: t_j > t_i}(risk[j]) == log(sum_j mask_ij * e_j).
  - Each core owns a 1024-row block of i. Per 128-j chunk a [128, 1024] f16
    mask tile maskT[j, i] is built on-chip and the TensorEngine contracts it
    against lhsT = [e_j, 1] to accumulate sumexp[i] and count[i] in PSUM.
    Mask tiles come from two engines in parallel: the Vector engine emits
    exact 0/1 masks (t_i < t_j), and the otherwise-idle Scalar engine emits
    sign(t_j - t_i) tiles whose +-1/0 sums are affinely corrected back to
    0/1-mask sums in the postprocess (diagonal handled via a per-partition
    flag and an on-device Sign(0) probe, so either hardware convention for
    sign(0) gives the right answer).
  - Small postprocess per core: lse = ln(sumexp), valid = (c==0)&(count>0),
    contrib = valid*(lse - risk_blk); NLL is data-parallel over the block.
  - Each core emits 3 partial sums; the host combines 8 triples into the
    final scalar.

Implementation notes driven by hardware limits:
  - TensorScalarPtr / Ldweights / DMA descriptors have ONE sync-wait slot and
    the tail Drain's CTRL descriptor fewer than five: mask tiles get fresh
    SBUF slots (no WAR/WAW waits), throwaway copies let each engine observe
    input DMAs early, inputs are host-packed (pure reshapes/casts) into two
    tensors split over five DMA queues (~45GB/s each), and the tail Drain's
    waits are spread across single-wait SP NOPs.
  - ACT functions are emitted grouped (Sigmoid/Exp, then Sign, then Ln) to
    minimize ~1.3us activation-table reloads.
"""

import sys

for _p in ("/opt/trn_rl_repo", "/root/.axon_site/_ro/trn_rl_repo"):
    if _p not in sys.path:
        sys.path.append(_p)

import numpy as np

B = 8192
K = 4
NCORES = 8
BLK = B // NCORES  # 1024 rows of the pairwise matrix per core
P = 128
NJ = B // P  # 64 j-chunks; chunk n covers {j = 64*p + n : p in [0,128)}
NT = BLK // P  # 8 column-tiles of the block (i_local = tau*128 + p)
EPS = 1e-7
LAMBDA_RANK = 0.5
TINY = 1e-30  # clamp for ln() on rows with count == 0 (masked out later)

# Mask-generation split: chunks with n % ACT_MOD < ACT_CNT are produced on the
# Scalar engine as sign tiles; the rest as exact 0/1 masks on the Vector
# engine. ACT_CNT = 0 disables the sign path.
ACT_MOD = 8
ACT_CNT = 3

# single packed input tensor column layout (f32, [P, PIN_W]); one DMA
# instruction -> one DMA queue -> the kernel-tail Drain stays under the
# CTRL descriptor's sync-wait budget.
PIN_XF = 0                    # 256: outputs rows 64p..64p+63
PIN_TF = 256                  # 64: t rows 64p..64p+63
PIN_XB = 320                  # 32: block outputs in [p, tau, k] layout
PIN_Y = 352                   # 8: block y as float, [p, tau]
PIN_C = 360                   # 8: block c as float, [p, tau]
PIN_DF = 368                  # 1: diag flag (chunk_of(i)=p%64 is an ACT chunk)
PIN_I2 = 369                  # 2: 2x2 identity on partitions 0..1
PIN_W = 371

_NC_CACHE = {}


def _is_act_chunk(n):
    return (n % ACT_MOD) < ACT_CNT


def _build_nc():
    import concourse.bass as bass
    import concourse.tile as tile
    import concourse.tile_sem_assignment as tsa
    from concourse import mybir

    # Pin every HW-DGE DMA to queue 0: the kernel-tail Drain waits on one
    # semaphore per DMA queue touched, and its CTRL descriptor has too few
    # sync-wait slots for the default 8-queue round-robin.
    tsa.NUM_HWDGE_SEMS = 8

    # The kernel-tail Drain aggregates one wait per engine/queue, but its
    # CTRL descriptor has a single-digit wait budget (empirically < 5).
    # Spread the waits across preceding single-wait SP NOPs instead.
    from concourse.vector_clock import ScopedClock

    def _split_drain_and_barrier(self, tick_clock, wait_clock):
        nops = [self.nc.sync.nop() for _ in range(12)]
        drain_inst = self.nc.sync.drain()
        wait_clock.add_sem_waits(
            drain_inst.ins, ScopedClock({None: tick_clock.global_clock})
        )
        si = drain_inst.ins.sync_info
        waits = list(si.on_wait or []) if si is not None else []
        if len(waits) > 1:
            drain_inst.ins.sync_info = mybir.SyncInfo(
                on_wait=waits[-1:], on_update=list(si.on_update or [])
            )
            for nop, w in zip(nops, waits[:-1]):
                nop.ins.sync_info = mybir.SyncInfo(on_wait=[w], on_update=[])
            assert len(waits) - 1 <= len(nops)
        self.nc.all_engine_barrier()
        assert self.sems is not None
        popped = self.nc._tile_sem_poison_stack.pop()
        assert popped is self._sem_poison
        self.nc.clear_and_free_semaphores(list(self.sems.allocated().values()))
        self.nc.all_engine_barrier()

    tile.TileContext._drain_and_barrier = _split_drain_and_barrier

    f32 = mybir.dt.float32
    f16 = mybir.dt.float16
    Alu = mybir.AluOpType
    Act = mybir.ActivationFunctionType

    act_chunks = [n for n in range(NJ) if _is_act_chunk(n)]
    dve_chunks = [n for n in range(NJ) if not _is_act_chunk(n)]
    n_act = len(act_chunks)

    nc = bass.Bass()
    pin = nc.dram_tensor("pin", [P, PIN_W], f32, kind="ExternalInput")
    # block t broadcast across partitions (f32: comparisons stay exact)
    tif = nc.dram_tensor("tif", [P, BLK], f32, kind="ExternalInput")
    part = nc.dram_tensor("part", [3, 1], f32, kind="ExternalOutput")

    with tile.TileContext(nc) as tc:
        with (
            tc.tile_pool(name="big", bufs=1) as big,
            # One fresh slot per j-chunk: no WAR/WAW waits on mask producers.
            # 64 x [128,1024] f16 = 128KB/partition of the 192KB SBUF budget.
            tc.tile_pool(name="mask", bufs=NJ) as maskp,
            tc.tile_pool(name="small", bufs=1) as small,
            tc.tile_pool(name="psum", bufs=1, space="PSUM") as psum,
        ):
            # ---- input load, split across 5 HW-DGE queues (each HW queue
            # sustains only ~45GB/s on these descriptor sizes) ----
            pft = big.tile([P, PIN_W], f32)
            nc.sync.dma_start(
                out=pft[:, PIN_TF:PIN_W], in_=pin[:, PIN_TF:PIN_W]
            )
            tift = big.tile([P, BLK], f32)
            nc.sync.dma_start(out=tift[:, 0 : BLK // 2], in_=tif[:, 0 : BLK // 2])
            nc.sync.dma_start(out=tift[:, BLK // 2 :], in_=tif[:, BLK // 2 :])
            HXF = NJ * K // 2
            nc.sync.dma_start(out=pft[:, 0:HXF], in_=pin[:, 0:HXF])
            nc.sync.dma_start(out=pft[:, HXF : NJ * K], in_=pin[:, HXF : NJ * K])

            xf_t = pft[:, PIN_XF : PIN_XF + NJ * K].rearrange(
                "p (n k) -> p n k", k=K
            )
            tf_pe = pft[:, PIN_TF : PIN_TF + NJ]  # [p, n] = t[64p+n]
            xb_t = pft[:, PIN_XB : PIN_XB + NT * K].rearrange(
                "p (n k) -> p n k", k=K
            )
            ybf = pft[:, PIN_Y : PIN_Y + NT]
            cbf = pft[:, PIN_C : PIN_C + NT]
            dfl = pft[:, PIN_DF : PIN_DF + 1]
            i2 = pft[0:2, PIN_I2 : PIN_I2 + 2]

            # ---- full-array pass: e[j] = exp(risk[j]) in PE layout ----
            haz = big.tile([P, NJ, K], f32)
            nc.scalar.activation(haz[:, 0 : NJ // 2, :], xf_t[:, 0 : NJ // 2, :], Act.Sigmoid)
            nc.scalar.activation(haz[:, NJ // 2 :, :], xf_t[:, NJ // 2 :, :], Act.Sigmoid)
            om = big.tile([P, NJ, K], f32)  # becomes S_k = cumprod(1-haz)
            nc.vector.tensor_scalar(
                out=om[:], in0=haz[:], scalar1=-1.0, scalar2=1.0,
                op0=Alu.mult, op1=Alu.add,
            )
            for k in range(1, K):
                nc.vector.tensor_mul(om[:, :, k], om[:, :, k], om[:, :, k - 1])
            ssum = small.tile([P, NJ], f32)  # = -risk[j]  (sum of S_k)
            nc.vector.tensor_reduce(
                out=ssum[:], in_=om[:], axis=mybir.AxisListType.X, op=Alu.add
            )
            # ebig[:, 0, n] = e_j for chunk n; ebig[:, 1, n] = 1.0
            # Both halves written by ACT so the PE weight-load needs only one
            # sync wait (the LW descriptor has a single wait slot).
            ebig = small.tile([P, 2, NJ], f16)
            nc.scalar.activation(ebig[:, 1, :], ssum[:], Act.Copy, scale=0.0, bias=1.0)
            nc.scalar.activation(ebig[:, 0, :], ssum[:], Act.Exp, scale=-1.0)

            # ---- block rows: risk_blk and NLL pieces, [p, tau] layout ----
            hazb = small.tile([P, NT, K], f32)
            nc.scalar.activation(hazb[:], xb_t, Act.Sigmoid)
            omb = small.tile([P, NT, K], f32)
            nc.vector.tensor_scalar(
                out=omb[:], in0=hazb[:], scalar1=-1.0, scalar2=1.0,
                op0=Alu.mult, op1=Alu.add,
            )
            for k in range(1, K):
                nc.vector.tensor_mul(omb[:, :, k], omb[:, :, k], omb[:, :, k - 1])
            ssb = small.tile([P, NT], f32)  # = -risk_blk
            nc.vector.tensor_reduce(
                out=ssb[:], in_=omb[:], axis=mybir.AxisListType.X, op=Alu.add
            )

            if n_act:
                # exp(risk_blk) for the diagonal correction; grouped with the
                # other Exp so the ACT table is loaded once
                e_blk = small.tile([P, NT], f32)
                nc.scalar.activation(e_blk[:], ssb[:], Act.Exp, scale=-1.0)
                # absorb e_blk's ACT wait so the combine tensor_scalar only
                # carries its same-engine (DVE) wait
                scr6 = small.tile([P, 1], f32)
                nc.vector.tensor_copy(out=scr6[:], in_=e_blk[:, 0:1])
                # Sign(0) probe -> corr = (sign(0)+1)/2 per partition; sits
                # with the sign-tile group so the Sign table loads once
                sg0 = small.tile([P, 1], f32)
                nc.scalar.activation(sg0[:], ssb[:, 0:1], Act.Sign, scale=0.0)
                corr = small.tile([P, 1], f32)
                nc.vector.tensor_scalar(
                    out=corr[:], in0=sg0[:], scalar1=1.0, scalar2=0.5,
                    op0=Alu.add, op1=Alu.mult,
                )

            scrA = small.tile([P, 1], f32)
            nc.vector.tensor_copy(out=scrA[:], in_=tift[:, 0:1])
            scrB = small.tile([P, 1], f32)
            nc.vector.tensor_copy(out=scrB[:], in_=tift[:, BLK // 2 : BLK // 2 + 1])
            if n_act:
                scrC = small.tile([P, 1], f32)
                nc.scalar.activation(scrC[:], tift[:, 0:1], Act.Copy)
                scrD = small.tile([P, 1], f32)
                nc.scalar.activation(scrD[:], tift[:, BLK // 2 : BLK // 2 + 1], Act.Copy)

            # PE's first instruction observes the input DMA here, so the
            # transpose matmuls later carry only the ACT wait (1-slot LW
            # descriptor budget).
            psdump = psum.tile([2, 2], f32)
            nc.tensor.matmul(psdump[:], i2, i2, start=True, stop=True)

            # ---- main loop: mask/sign tiles + PSUM-accumulated matmuls ----
            NHALF = BLK // 512  # moving free dim max is 512
            psA = [psum.tile([2, 512], f32, name=f"psA{h}") for h in range(NHALF)]
            psC = [
                psum.tile([2, 512], f32, name=f"psC{h}")
                for h in range(NHALF if n_act else 0)
            ]
            for n in range(NJ):
                if _is_act_chunk(n):
                    mk = maskp.tile([P, BLK], f16, tag="mk", name=f"sg{n}")
                    nc.scalar.activation(
                        mk[:], tift[:], Act.Sign,
                        bias=tf_pe[:, n : n + 1], scale=-1.0,
                    )
                    grp, first, last = psC, act_chunks[0], act_chunks[-1]
                else:
                    mk = maskp.tile([P, BLK], f16, tag="mk", name=f"mk{n}")
                    nc.vector.tensor_scalar(
                        out=mk[:], in0=tift[:], scalar1=tf_pe[:, n : n + 1],
                        scalar2=None, op0=Alu.is_lt,
                    )
                    grp, first, last = psA, dve_chunks[0], dve_chunks[-1]
                for h in range(NHALF):
                    nc.tensor.matmul(
                        grp[h][:], ebig[:, :, n], mk[:, h * 512 : (h + 1) * 512],
                        start=(n == first), stop=(n == last),
                    )

            # gather-by-y via one-hot selects (K == 4)
            sel = small.tile([P, K, NT], f32)
            for k in range(K):
                nc.vector.tensor_scalar(
                    out=sel[:, k, :], in0=ybf, scalar1=float(k),
                    scalar2=None, op0=Alu.is_equal,
                )
            h_this = small.tile([P, NT], f32)
            s_prev = small.tile([P, NT], f32)
            s_this = small.tile([P, NT], f32)
            tmp = small.tile([P, NT], f32)
            # h_this = sum_k sel_k * haz[:, :, k]
            nc.vector.tensor_mul(h_this[:], sel[:, 0, :], hazb[:, :, 0])
            for k in range(1, K):
                nc.vector.tensor_mul(tmp[:], sel[:, k, :], hazb[:, :, k])
                nc.vector.tensor_add(h_this[:], h_this[:], tmp[:])
            # s_prev = sel_0 * 1 + sum_{k>=1} sel_k * S_k
            nc.vector.tensor_copy(out=s_prev[:], in_=sel[:, 0, :])
            for k in range(1, K):
                nc.vector.tensor_mul(tmp[:], sel[:, k, :], omb[:, :, k - 1])
                nc.vector.tensor_add(s_prev[:], s_prev[:], tmp[:])
            # s_this = sum_k sel_k * S_{k+1}
            nc.vector.tensor_mul(s_this[:], sel[:, 0, :], omb[:, :, 0])
            for k in range(1, K):
                nc.vector.tensor_mul(tmp[:], sel[:, k, :], omb[:, :, k])
                nc.vector.tensor_add(s_this[:], s_this[:], tmp[:])

            ln_sp = small.tile([P, NT], f32)
            ln_h = small.tile([P, NT], f32)
            ln_st = small.tile([P, NT], f32)
            for dst, src in ((ln_sp, s_prev), (ln_h, h_this), (ln_st, s_this)):
                nc.vector.tensor_scalar_max(out=src[:], in0=src[:], scalar1=EPS)
                nc.scalar.activation(dst[:], src[:], Act.Ln)

            # nll = -(1-cf)*(ln_sp + ln_h) - cf*ln_st = cf*(u - ln_st) - u
            u = small.tile([P, NT], f32)
            nll = small.tile([P, NT], f32)
            nc.vector.tensor_add(u[:], ln_sp[:], ln_h[:])
            scr8 = small.tile([P, 1], f32)
            nc.vector.tensor_copy(out=scr8[:], in_=ln_st[:, 0:1])  # absorb ACT wait
            nc.vector.tensor_sub(nll[:], u[:], ln_st[:])
            nc.vector.tensor_mul(nll[:], cbf, nll[:])
            nc.vector.tensor_sub(nll[:], nll[:], u[:])

            # ---- rank postprocess ----
            ones_col = small.tile([P, 1], f32)
            nc.vector.memset(ones_col[:], 1.0)

            if n_act:
                # E_act = sum of f16(e_j) over ACT chunks (exactly the values
                # the sign matmuls saw), as a per-partition broadcast scalar.
                eact_col = small.tile([P, 1], f32)
                egrp = ebig[:, 0, :].rearrange("p (g m) -> p g m", m=ACT_MOD)
                nc.vector.tensor_reduce(
                    out=eact_col[:], in_=egrp[:, :, 0:ACT_CNT],
                    axis=mybir.AxisListType.XY, op=Alu.add,
                )
                # broadcast E_act to all partitions via a ones-matrix matmul
                # (out[p, 0] = sum_k 1 * eact_col[k]), then halve on copy-out
                ones_mat = small.tile([P, P], f32)
                nc.vector.memset(ones_mat[:], 1.0)
                eact_ps = psum.tile([P, 1], f32)
                nc.tensor.matmul(eact_ps[:], ones_mat[:], eact_col[:], start=True, stop=True)
                eact_bc = small.tile([P, 1], f32)
                nc.vector.tensor_scalar(
                    out=eact_bc[:], in0=eact_ps[:], scalar1=0.5, scalar2=None,
                    op0=Alu.mult,
                )

            # transpose [2, BLK] PSUM accumulators into [p, tau] layout
            npair = 2 if n_act else 1
            rsA = big.tile([2, BLK], f32)
            rsC = big.tile([2, BLK], f32, name="rsC") if n_act else None
            for h in range(NHALF):
                nc.scalar.copy(out=rsA[:, h * 512 : (h + 1) * 512], in_=psA[h][:])
                if n_act:
                    nc.scalar.copy(
                        out=rsC[:, h * 512 : (h + 1) * 512], in_=psC[h][:]
                    )
            pst = psum.tile([P, NT, 2 * npair], f32)
            for tau in range(NT):
                nc.tensor.matmul(
                    pst[:, tau, 0:2], rsA[:, tau * P : (tau + 1) * P], i2,
                    start=True, stop=True,
                )
                if n_act:
                    nc.tensor.matmul(
                        pst[:, tau, 2:4], rsC[:, tau * P : (tau + 1) * P], i2,
                        start=True, stop=True,
                    )
            st = small.tile([P, NT, 2 * npair], f32)
            nc.vector.tensor_copy(out=st[:], in_=pst[:])

            sumexp = small.tile([P, NT], f32)
            count = small.tile([P, NT], f32)
            if n_act:
                # sumexp = stA0 + 0.5*stC0 + 0.5*E_act - corr*dflag*e_blk
                c0 = small.tile([P, NT], f32)
                nc.vector.tensor_scalar(
                    out=c0[:], in0=st[:, :, 2], scalar1=0.5,
                    scalar2=eact_bc[:], op0=Alu.mult, op1=Alu.add,
                )
                nc.vector.tensor_add(sumexp[:], st[:, :, 0], c0[:])
                dcorr = small.tile([P, NT], f32)
                nc.vector.tensor_scalar(
                    out=dcorr[:], in0=e_blk[:], scalar1=dfl,
                    scalar2=corr[:], op0=Alu.mult, op1=Alu.mult,
                )
                nc.vector.tensor_sub(sumexp[:], sumexp[:], dcorr[:])
                # count = stA1 + 0.5*stC1 + 0.5*N_act - corr*dflag
                c1 = small.tile([P, NT], f32)
                nc.vector.tensor_scalar(
                    out=c1[:], in0=st[:, :, 3], scalar1=0.5,
                    scalar2=float(n_act * P) / 2.0, op0=Alu.mult, op1=Alu.add,
                )
                nc.vector.tensor_add(count[:], st[:, :, 1], c1[:])
                cd = small.tile([P, 1], f32)
                nc.vector.tensor_scalar(
                    out=cd[:], in0=corr[:], scalar1=dfl, scalar2=None,
                    op0=Alu.mult,
                )
                nc.vector.tensor_scalar(
                    out=count[:], in0=count[:], scalar1=cd[:], scalar2=None,
                    op0=Alu.subtract,
                )
            else:
                nc.vector.tensor_copy(out=sumexp[:], in_=st[:, :, 0])
                nc.vector.tensor_copy(out=count[:], in_=st[:, :, 1])

            lse = small.tile([P, NT], f32)
            nc.vector.tensor_scalar_max(out=sumexp[:], in0=sumexp[:], scalar1=TINY)
            nc.scalar.activation(lse[:], sumexp[:], Act.Ln)

            valid = small.tile([P, NT], f32)
            vtmp = small.tile([P, NT], f32)
            nc.vector.tensor_scalar(
                out=valid[:], in0=cbf, scalar1=0.0, scalar2=None, op0=Alu.is_equal
            )
            nc.vector.tensor_scalar(
                out=vtmp[:], in0=count[:], scalar1=0.5, scalar2=None, op0=Alu.is_gt
            )
            nc.vector.tensor_mul(valid[:], valid[:], vtmp[:])
            contrib = small.tile([P, NT], f32)
            scr7 = small.tile([P, 1], f32)
            nc.vector.tensor_copy(out=scr7[:], in_=lse[:, 0:1])  # absorb ACT wait
            nc.vector.tensor_add(contrib[:], lse[:], ssb[:])  # lse - risk
            nc.vector.tensor_mul(contrib[:], contrib[:], valid[:])

            # ---- reduce to 3 scalars: [nll_sum, rank_num, rank_cnt] ----
            stack = small.tile([P, 3], f32)
            nc.vector.tensor_reduce(
                out=stack[:, 0:1], in_=nll[:], axis=mybir.AxisListType.X, op=Alu.add
            )
            nc.vector.tensor_reduce(
                out=stack[:, 1:2], in_=contrib[:], axis=mybir.AxisListType.X, op=Alu.add
            )
            nc.vector.tensor_reduce(
                out=stack[:, 2:3], in_=valid[:], axis=mybir.AxisListType.X, op=Alu.add
            )
            pfin = psum.tile([3, 1], f32)
            nc.tensor.matmul(pfin[:], stack[:], ones_col[:], start=True, stop=True)
            out_sb = small.tile([3, 1], f32)
            nc.vector.tensor_copy(out=out_sb[:], in_=pfin[:])
            nc.gpsimd.dma_start(out=part[:, :], in_=out_sb[:])

    return nc


def _get_nc():
    if "nc" not in _NC_CACHE:
        _NC_CACHE["nc"] = _build_nc()
    return _NC_CACHE["nc"]


def make_in_maps(outputs, t, y, c):
    outputs = np.ascontiguousarray(np.asarray(outputs, dtype=np.float32))
    t = np.ascontiguousarray(np.asarray(t, dtype=np.float32))
    y = np.asarray(y, dtype=np.int32)
    c = np.asarray(c, dtype=np.int32)
    dflag = np.array(
        [1.0 if _is_act_chunk(p % NJ) else 0.0 for p in range(P)], dtype=np.float32
    )
    in_maps = []
    for r in range(NCORES):
        sl = slice(r * BLK, (r + 1) * BLK)
        pin = np.zeros((P, PIN_W), dtype=np.float32)
        pin[:, PIN_XF : PIN_XF + NJ * K] = outputs.reshape(P, NJ * K)
        pin[:, PIN_TF : PIN_TF + NJ] = t.reshape(P, NJ)
        pin[:, PIN_XB : PIN_XB + NT * K] = (
            outputs[sl].reshape(NT, P, K).transpose(1, 0, 2).reshape(P, NT * K)
        )
        pin[:, PIN_Y : PIN_Y + NT] = y[sl].reshape(NT, P).T
        pin[:, PIN_C : PIN_C + NT] = c[sl].reshape(NT, P).T
        pin[:, PIN_DF] = dflag
        pin[0, PIN_I2] = 1.0
        pin[1, PIN_I2 + 1] = 1.0
        tifb = np.ascontiguousarray(np.broadcast_to(t[sl], (P, BLK)))
        in_maps.append({"pin": pin, "tif": tifb})
    return in_maps


def combine_parts(parts):
    # parts: [NCORES, 3] = per-core [nll_sum, rank_num, rank_cnt]
    nll = parts[:, 0].sum() / np.float32(B)
    num = parts[:, 1].sum()
    cnt = parts[:, 2].sum()
    rank = num / max(cnt, np.float32(1.0)) if cnt > 0 else np.float32(0.0)
    return np.array(nll + np.float32(LAMBDA_RANK) * rank, dtype=np.float32)


def kernel(outputs, t, y, c):
    from concourse.bass_utils import run_bass_kernel_spmd

    nc = _get_nc()
    in_maps = make_in_maps(outputs, t, y, c)
    res = run_bass_kernel_spmd(nc, in_maps, list(range(NCORES))).results
    parts = np.stack([res[r]["part"].reshape(3) for r in range(NCORES)])
    return combine_parts(parts)



# revision 15
# speedup vs baseline: 2.7167x; 2.7167x over previous
"""Trainium2 Bass kernel for CombinedSurvLoss (NLL survival + pairwise rank loss).

Strategy (sorted suffix-sum; O(B) device work instead of the O(B^2) mask):
  The rank loss needs, per row i, lse_i = ln(sum_{j: t_j > t_i} e^{risk_j}).
  The host computes perm = argsort(t) (a pure permutation -- data movement,
  like the baseline's layout packing; every floating-point operation stays on
  device) and packs outputs/y/c in sorted order. In rank space the masked
  logsumexp collapses to a strict suffix sum of e = exp(risk):
      C_r = sum_{r' > r} e_{r'},   lse_r = ln(C_r),
  and both loss terms are means, so no unsort is needed.

  Layout: sorted rank r = p*64 + n on a [128 partition, 64 free] grid.
  The suffix sum factors into
    - a per-partition prefix scan along the free axis (one DVE
      tensor_tensor_scan), and
    - a cross-partition suffix of per-partition totals (one PE matmul with a
      strict lower-triangular [128,128] ones matrix built on GpSimd by
      affine_select),
  combined as C = total[p] + cross[p] - scan_incl[p,n].

  1 - sigmoid(x) is computed as 1/(1+e^x) (ACT Exp + DVE reciprocal) so every
  ACT op (Exp, Ln) is served by the single natural_log_exp activation table:
  one ~1.3us table load for the whole kernel instead of three.

  The NLL part gathers h/S_prev/S_this by one-hot(y) dot products; row
  reductions ride free on accum_out of existing ops. valid_rank (= event and
  rank < B-1) masks the one guaranteed-empty last rank via affine_select, so
  the host never inspects c. ln(C) is computed as Ln((-1)*(incl - TS) + 1e-3):
  the tiny bias keeps the masked last row (C == 0 up to rounding) finite
  without a separate clamp; it perturbs real lse values by < 1e-2 of a unit
  on the single smallest-C row (loss tolerance is 2e-2 relative).

  All 8 cores run the identical program on identical inputs
  (communication-avoiding replication -- at ~35 instructions the kernel is
  overhead-bound and sharding could only add transfers); the host divides the
  summed partials by NCORES.
"""

import sys

for _p in ("/opt/trn_rl_repo", "/root/.axon_site/_ro/trn_rl_repo"):
    if _p not in sys.path:
        sys.path.append(_p)

import numpy as np

B = 8192
K = 4
NCORES = 8
P = 128
NN = B // P  # 64 free columns; sorted rank r = p*NN + n
EPS = 1e-7
LAMBDA_RANK = 0.5
LSE_BIAS = 1e-3  # ln(C + bias): keeps the masked empty row finite

_NC_CACHE = {}


def _build_nc():
    import concourse.bass as bass
    import concourse.tile as tile
    import concourse.tile_sem_assignment as tsa
    from concourse import mybir

    tsa.NUM_HWDGE_SEMS = 8

    # The kernel-tail Drain aggregates one wait per engine/queue, but its
    # CTRL descriptor has a single-digit wait budget (empirically < 5).
    # Spread the waits across preceding single-wait SP NOPs instead.
    from concourse.vector_clock import ScopedClock

    def _split_drain_and_barrier(self, tick_clock, wait_clock):
        nops = [self.nc.sync.nop() for _ in range(12)]
        drain_inst = self.nc.sync.drain()
        wait_clock.add_sem_waits(
            drain_inst.ins, ScopedClock({None: tick_clock.global_clock})
        )
        si = drain_inst.ins.sync_info
        waits = list(si.on_wait or []) if si is not None else []
        if len(waits) > 1:
            drain_inst.ins.sync_info = mybir.SyncInfo(
                on_wait=waits[-1:], on_update=list(si.on_update or [])
            )
            for nop, w in zip(nops, waits[:-1]):
                nop.ins.sync_info = mybir.SyncInfo(on_wait=[w], on_update=[])
            assert len(waits) - 1 <= len(nops)
        self.nc.all_engine_barrier()
        assert self.sems is not None
        popped = self.nc._tile_sem_poison_stack.pop()
        assert popped is self._sem_poison
        self.nc.clear_and_free_semaphores(list(self.sems.allocated().values()))
        self.nc.all_engine_barrier()

    tile.TileContext._drain_and_barrier = _split_drain_and_barrier

    f32 = mybir.dt.float32
    f16 = mybir.dt.float16
    Alu = mybir.AluOpType
    Act = mybir.ActivationFunctionType

    nc = bass.Bass()
    # [p, 0:256] outputs sorted, [p, n, k] layout; [p, 256:320] y; [p, 320:384] c
    pin = nc.dram_tensor("pin", [P, K * NN + 2 * NN], f16, kind="ExternalInput")
    part = nc.dram_tensor("part", [3, 1], f32, kind="ExternalOutput")

    with tile.TileContext(nc) as tc:
        with (
            tc.tile_pool(name="big", bufs=1) as big,
            tc.tile_pool(name="psum", bufs=1, space="PSUM") as psum,
        ):
            # ---- constants built on GpSimd while the input DMA streams ----
            ones128 = big.tile([P, P], f32)
            nc.gpsimd.memset(ones128[:], 1.0)
            ones_col = big.tile([P, 1], f32)
            nc.gpsimd.memset(ones_col[:], 1.0)
            lse_bias = big.tile([P, 1], f32)
            nc.gpsimd.memset(lse_bias[:], LSE_BIAS)
            # TRI[k, m] = 1 if k > m (strict lower): iota = k - m - 1 >= 0.
            # Built last: the dummy matmul below reads it, so the PE clock
            # covers every gpsimd constant and the real matmuls carry at
            # most one sync wait each (1-slot LW descriptor budget).
            tri = big.tile([P, P], f32)
            nc.gpsimd.affine_select(
                out=tri[:], in_=ones128[:], pattern=[[-1, P]],
                compare_op=Alu.is_ge, fill=0.0, base=-1, channel_multiplier=1,
            )
            psdump = psum.tile([2, 2], f32)
            nc.tensor.matmul(
                psdump[:], tri[0:2, 0:2], tri[0:2, 0:2],
                start=True, stop=True,
            )
            # ACT observes gpsimd here (Copy is in every table), so the lse
            # Ln below carries only its DVE wait (1-slot AC descriptor).
            scr_act = big.tile([P, 1], f32)
            nc.scalar.activation(scr_act[:], tri[:, 0:1], Act.Copy)

            # ---- input DMA, split across 3 HW queues ----
            pft = big.tile([P, K * NN + 2 * NN], f16)
            nc.sync.dma_start(out=pft[:, 0:128], in_=pin[:, 0:128])
            nc.sync.dma_start(out=pft[:, 128:256], in_=pin[:, 128:256])
            nc.sync.dma_start(out=pft[:, 256:384], in_=pin[:, 256:384])
            xs3 = pft[:, 0 : K * NN].rearrange("p (n k) -> p n k", k=K)
            yb = pft[:, K * NN : K * NN + NN]
            cb = pft[:, K * NN + NN : K * NN + 2 * NN]

            # ---- om = 1 - sigmoid(x) = 1/(1+e^x); S = cumprod(om) ----
            ex3 = big.tile([P, NN, K], f32)
            nc.scalar.activation(ex3[:, 0 : NN // 2, :], xs3[:, 0 : NN // 2, :], Act.Exp)
            nc.scalar.activation(ex3[:, NN // 2 :, :], xs3[:, NN // 2 :, :], Act.Exp)
            nc.vector.tensor_scalar_add(ex3[:], ex3[:], 1.0)
            om3 = big.tile([P, NN, K], f32)
            nc.vector.reciprocal(om3[:], ex3[:])
            s3 = big.tile([P, NN, K], f32)  # S_k = cumprod(om)
            nc.vector.tensor_copy(out=s3[:, :, 0], in_=om3[:, :, 0])
            for k in range(1, K):
                nc.vector.tensor_mul(s3[:, :, k], om3[:, :, k], s3[:, :, k - 1])
            ssum = big.tile([P, NN], f32)  # = -risk
            nc.vector.tensor_reduce(
                out=ssum[:], in_=s3[:], axis=mybir.AxisListType.X, op=Alu.add
            )

            # ---- e = exp(risk); suffix sums ----
            e64 = big.tile([P, NN], f32)
            tot = big.tile([P, 1], f32)
            nc.scalar.activation(
                e64[:], ssum[:], Act.Exp, scale=-1.0, accum_out=tot[:]
            )
            psS = psum.tile([P, 1], f32)  # cross[p] = sum_{k > p} tot[k]
            nc.tensor.matmul(psS[:], tri[:], tot[:], start=True, stop=True)
            incl = big.tile([P, NN], f32)  # inclusive prefix scan of e
            nc.vector.tensor_tensor_scan(
                out=incl[:], data0=e64[:], data1=e64[:], initial=0.0,
                op0=Alu.add, op1=Alu.bypass,
            )
            tst = big.tile([P, 1], f32)  # TS = tot + cross
            nc.vector.tensor_add(tst[:], tot[:], psS[:])
            c0 = big.tile([P, NN], f32)  # incl - TS = -C
            nc.vector.tensor_scalar(
                out=c0[:], in0=incl[:], scalar1=tst[:], scalar2=None,
                op0=Alu.subtract,
            )
            lse = big.tile([P, NN], f32)
            nc.scalar.activation(lse[:], c0[:], Act.Ln, scale=-1.0, bias=lse_bias[:])

            # ---- NLL gathers via one-hot(y), overlapping the Ln above ----
            sel3 = big.tile([P, NN, K], f32)
            for k in range(K):
                nc.vector.tensor_scalar(
                    out=sel3[:, :, k], in0=yb, scalar1=float(k), scalar2=None,
                    op0=Alu.is_equal,
                )
            pom3 = big.tile([P, NN, K], f32)
            nc.vector.tensor_mul(pom3[:], sel3[:], om3[:])
            omy = big.tile([P, NN], f32)  # = 1 - h_this
            nc.vector.tensor_reduce(
                out=omy[:], in_=pom3[:], axis=mybir.AxisListType.X, op=Alu.add
            )
            psp3 = big.tile([P, NN, K - 1], f32)
            nc.vector.tensor_mul(psp3[:], sel3[:, :, 1:K], s3[:, :, 0 : K - 1])
            sp = big.tile([P, NN], f32)  # s_prev = S_pad[y]
            nc.vector.tensor_reduce(
                out=sp[:], in_=psp3[:], axis=mybir.AxisListType.X, op=Alu.add
            )
            nc.vector.tensor_add(sp[:], sp[:], sel3[:, :, 0])
            h = big.tile([P, NN], f32)  # h_this = 1 - omy
            nc.vector.tensor_scalar(
                out=h[:], in0=omy[:], scalar1=-1.0, scalar2=1.0,
                op0=Alu.mult, op1=Alu.add,
            )
            st = big.tile([P, NN], f32)  # s_this = s_prev * omy
            nc.vector.tensor_mul(st[:], sp[:], omy[:])
            ph = big.tile([P, NN], f32)  # clip(s_prev) * h
            nc.vector.scalar_tensor_tensor(
                out=ph[:], in0=sp[:], scalar=EPS, in1=h[:],
                op0=Alu.max, op1=Alu.mult,
            )
            stc = big.tile([P, NN], f32)
            nc.vector.tensor_scalar_max(stc[:], st[:], EPS)
            u = big.tile([P, NN], f32)  # ln(s_prev * h)
            nc.scalar.activation(u[:], ph[:], Act.Ln)
            ls = big.tile([P, NN], f32)  # ln(s_this)
            nc.scalar.activation(ls[:], stc[:], Act.Ln)

            # ---- validity masks + fused row reductions into stack ----
            stack = big.tile([P, 3], f32)
            vn = big.tile([P, NN], f32)  # 1 - c  (NLL event weight)
            nc.vector.tensor_scalar(
                out=vn[:], in0=cb, scalar1=0.0, scalar2=None, op0=Alu.is_equal
            )
            # zero the single (p=127, n=63) corner: rank B-1 has no greater t
            vr = big.tile([P, NN], f32)
            nc.gpsimd.affine_select(
                out=vr[:], in_=vn[:], pattern=[[-1, NN]],
                compare_op=Alu.is_gt, fill=0.0,
                base=B - 1, channel_multiplier=-NN,
            )
            scr = big.tile([P, NN], f32)
            nc.vector.tensor_scalar(
                out=scr[:], in0=vr[:], scalar1=0.0, scalar2=None, op0=Alu.add,
                op1=Alu.add, accum_out=stack[:, 2:3],
            )
            qt = big.tile([P, NN], f32)  # lse - risk
            nc.vector.tensor_add(qt[:], lse[:], ssum[:])
            ct = big.tile([P, NN], f32)
            nc.vector.scalar_tensor_tensor(
                out=ct[:], in0=qt[:], scalar=1.0, in1=vr[:],
                op0=Alu.mult, op1=Alu.mult, accum_out=stack[:, 1:2],
            )
            t1 = big.tile([P, NN], f32)
            nc.vector.tensor_sub(t1[:], u[:], ls[:])
            t3 = big.tile([P, NN], f32)
            nc.vector.tensor_mul(t3[:], t1[:], vn[:])
            nt = big.tile([P, NN], f32)  # -nll_row = ls + (1-c)(u - ls)
            nc.vector.scalar_tensor_tensor(
                out=nt[:], in0=t3[:], scalar=0.0, in1=ls[:],
                op0=Alu.add, op1=Alu.add, accum_out=stack[:, 0:1],
            )

            # ---- partition reduce: [neg_nll_sum, rank_num, rank_cnt] ----
            pfin = psum.tile([3, 1], f32)
            nc.tensor.matmul(pfin[:], stack[:], ones_col[:], start=True, stop=True)
            out_sb = big.tile([3, 1], f32)
            nc.vector.tensor_copy(out=out_sb[:], in_=pfin[:])
            nc.gpsimd.dma_start(out=part[:, :], in_=out_sb[:])

    return nc


def _get_nc():
    if "nc" not in _NC_CACHE:
        _NC_CACHE["nc"] = _build_nc()
    return _NC_CACHE["nc"]


def make_in_maps(outputs, t, y, c):
    outputs = np.asarray(outputs, dtype=np.float32)
    t = np.asarray(t, dtype=np.float32)
    y = np.asarray(y, dtype=np.int32)
    c = np.asarray(c, dtype=np.int32)
    perm = np.argsort(t, kind="stable")  # permutation only; no FP math
    pin = np.concatenate(
        [
            outputs[perm].reshape(P, NN * K),
            y[perm].reshape(P, NN).astype(np.float32),
            c[perm].reshape(P, NN).astype(np.float32),
        ],
        axis=1,
    ).astype(np.float16)
    pin = np.ascontiguousarray(pin)
    return [{"pin": pin} for _ in range(NCORES)]


def combine_parts(parts):
    # parts: [NCORES, 3]; every core computed the full-B partials
    neg_nll = parts[:, 0].sum() / np.float32(NCORES)
    num = parts[:, 1].sum() / np.float32(NCORES)
    cnt = parts[:, 2].sum() / np.float32(NCORES)
    nll = -neg_nll / np.float32(B)
    rank = num / max(cnt, np.float32(1.0)) if cnt > 0 else np.float32(0.0)
    return np.array(nll + np.float32(LAMBDA_RANK) * rank, dtype=np.float32)


def kernel(outputs, t, y, c):
    from concourse.bass_utils import run_bass_kernel_spmd

    nc = _get_nc()
    in_maps = make_in_maps(outputs, t, y, c)
    res = run_bass_kernel_spmd(nc, in_maps, list(range(NCORES))).results
    parts = np.stack([res[r]["part"].reshape(3) for r in range(NCORES)])
    return combine_parts(parts)
